# revision 24
# baseline (speedup 1.0000x reference)
"""MultiHeadAttention Trainium2 kernel.

Sharding: 8 cores = 4 batches x 2 head-groups (8 heads each).
Per core: full attention for its (batch, 8-head group) slice plus its share
of the out-projection; host sums the two head-group partials per batch.

Orientation: everything transposed on-chip (x^T in, Q^T/K^T per dout-tile,
V natural [s,d] with an appended ones column so the attnV matmul also
produces softmax denominators). Softmax without max-subtraction (scores are
~N(0,1); exp is safe in fp32).
"""

import sys

import numpy as np

for _p in ("/opt/trn_rl_repo", "/root/.axon_site/_ro/trn_rl_repo"):
    if _p not in sys.path:
        sys.path.append(_p)

B, S, D = 4, 2048, 1024
H, HD = 16, 64
HPC = 8          # heads per core
DG = HPC * HD    # 512: dout slice per core per projection
N_CORES = 8

_cached = None


def _build():
    from concourse import bass, bacc, tile
    from concourse.mybir import dt, ActivationFunctionType as AF

    f32 = dt.float32
    f32r = dt.float32r

    nc = bacc.Bacc()
    xT_d = nc.declare_dram_parameter("xT", [D, S], f32r, isOutput=False)
    wq_d = nc.declare_dram_parameter("wq", [D, DG], f32r, isOutput=False)
    wk_d = nc.declare_dram_parameter("wk", [D, DG], f32r, isOutput=False)
    wv_d = nc.declare_dram_parameter("wv", [D, DG], f32r, isOutput=False)
    bq_d = nc.declare_dram_parameter("bq", [128, 4], f32, isOutput=False)
    bk_d = nc.declare_dram_parameter("bk", [128, 4], f32, isOutput=False)
    bv_d = nc.declare_dram_parameter("bv", [1, DG], f32, isOutput=False)
    wo_d = nc.declare_dram_parameter("wo", [DG, D], f32r, isOutput=False)
    ones_d = nc.declare_dram_parameter("ones", [1, 64], f32r, isOutput=False)
    y_d = nc.declare_dram_parameter("y", [S, D], f32, isOutput=True)

    with tile.TileContext(nc) as tc:
        with tc.tile_pool(name="persist", bufs=1) as persist:
            QT = persist.tile([128, 4, S], f32r)   # [dout%128, dout//128, s]
            KT = persist.tile([128, 4, S], f32r)
            Vn = persist.tile([128, 16, HPC, HD + 1], f32r)  # [s%128, s//128, h, d|1]
            bq_sb = persist.tile([128, 4], f32)
            bk_sb = persist.tile([128, 4], f32)
            bv_sb = persist.tile([128, HPC, HD], f32)
            ones_sb = persist.tile([1, 64], f32r)

            nc.sync.dma_start(bq_sb[:, :], bq_d[:, :])
            nc.sync.dma_start(bk_sb[:, :], bk_d[:, :])
            bv_bcast = bass.AP(
                tensor=bv_d, offset=0, ap=[[0, 128], [HD, HPC], [1, HD]]
            )
            nc.sync.dma_start(bv_sb[:, :, :], bv_bcast)
            nc.sync.dma_start(ones_sb[:, :], ones_d[:, :])
            vn1 = persist.tile([128, 16, HPC, 1], f32)
            nc.vector.memset(vn1[:, :, :, :], 1.0)
            nc.vector.tensor_copy(Vn[:, :, :, HD : HD + 1], vn1[:, :, :, :])

            # ---------------- phase 1: QKV projections ----------------
            with (
                tc.tile_pool(name="wpool", bufs=1) as wpool,
                tc.tile_pool(name="xtp", bufs=2) as xtp,
                tc.tile_pool(name="ps_qkv", bufs=4, space="PSUM") as ps_qkv,
            ):
                wq_sb = wpool.tile([128, 8, DG], f32r)
                wk_sb = wpool.tile([128, 8, DG], f32r)
                wv_sb = wpool.tile([128, 8, DG], f32r)
                for wsb, wd in ((wq_sb, wq_d), (wk_sb, wk_d), (wv_sb, wv_d)):
                    nc.sync.dma_start(
                        wsb[:, :, :],
                        bass.AP(tensor=wd, offset=0,
                                ap=[[DG, 128], [128 * DG, 8], [1, DG]]),
                    )

                for sc in range(4):  # s chunks of 512
                    xt = xtp.tile([128, 8, 512], f32r)
                    nc.sync.dma_start(
                        xt[:, :, :],
                        bass.AP(tensor=xT_d, offset=sc * 512,
                                ap=[[S, 128], [128 * S, 8], [1, 512]]),
                    )
                    for wsb, bsb, dstT in ((wq_sb, bq_sb, QT), (wk_sb, bk_sb, KT)):
                        for t in range(4):  # dout tiles
                            ps = ps_qkv.tile([128, 512], f32)
                            for di in range(8):
                                nc.tensor.matmul(
                                    ps[:, :],
                                    wsb[:, di, t * 128 : (t + 1) * 128],
                                    xt[:, di, :],
                                    start=(di == 0),
                                    stop=(di == 7),
                                )
                            nc.scalar.activation(
                                dstT[:, t, sc * 512 : (sc + 1) * 512],
                                ps[:, :],
                                AF.Identity,
                                bias=bsb[:, t : t + 1],
                            )
                    for st in range(4):  # s sub-tiles (V natural)
                        stile = sc * 4 + st
                        ps = ps_qkv.tile([128, HPC, HD], f32)
                        for di in range(8):
                            nc.tensor.matmul(
                                ps[:, :, :],
                                xt[:, di, st * 128 : (st + 1) * 128],
                                wv_sb[:, di, :],
                                start=(di == 0),
                                stop=(di == 7),
                            )
                        nc.vector.tensor_add(
                            Vn[:, stile, :, 0:HD], ps[:, :, :], bv_sb[:, :, :]
                        )

            # ---------------- phase 2: attention + out-projection ----------------
            with (
                tc.tile_pool(name="wout", bufs=1) as wopool,
                tc.tile_pool(name="expp", bufs=3) as expp,
                tc.tile_pool(name="attp", bufs=2) as attp,
                tc.tile_pool(name="recp", bufs=4) as recp,
                tc.tile_pool(name="bcp", bufs=2) as bcp,
                tc.tile_pool(name="yp", bufs=3) as yp,
                tc.tile_pool(name="ps_s", bufs=2, space="PSUM") as ps_sp,
                tc.tile_pool(name="ps_a", bufs=2, space="PSUM") as ps_ap,
                tc.tile_pool(name="ps_b", bufs=1, space="PSUM") as ps_bp,
                tc.tile_pool(name="ps_y", bufs=1, space="PSUM") as ps_yp,
            ):
                wo_sb = wopool.tile([128, 4, D], f32r)
                nc.sync.dma_start(
                    wo_sb[:, :, :],
                    bass.AP(tensor=wo_d, offset=0,
                            ap=[[D, 128], [128 * D, 4], [1, D]]),
                )

                for qc in range(4):  # q chunks of 512
                    attnT = attp.tile([128, 4, 512], f32r)  # [d%128, dtile, q]
                    for h in range(HPC):
                        tq = h // 2
                        pb = (h % 2) * 64
                        ps_att = ps_ap.tile([128, 512], f32)
                        for pair in range(8):  # 2 k-tiles per exp instruction
                            ps_s = ps_sp.tile([128, 1024], f32)
                            for j in range(2):
                                t = pair * 2 + j
                                nc.tensor.matmul(
                                    ps_s[:, j * 512 : (j + 1) * 512],
                                    KT[pb : pb + 64, tq, t * 128 : (t + 1) * 128],
                                    QT[pb : pb + 64, tq, qc * 512 : (qc + 1) * 512],
                                    start=True,
                                    stop=True,
                                )
                            ex = expp.tile([128, 1024], f32r)
                            nc.scalar.activation(ex[:, :], ps_s[:, :], AF.Exp)
                            for j in range(2):
                                t = pair * 2 + j
                                nc.tensor.matmul(
                                    ps_att[0:65, :],
                                    Vn[:, t, h, :],
                                    ex[:, j * 512 : (j + 1) * 512],
                                    start=(t == 0),
                                    stop=(t == 15),
                                )
                        rc = recp.tile([1, 512], f32r)
                        with nc.allow_low_precision(reason="fp32r is fp32-width"):
                            nc.vector.reciprocal(rc[:, :], ps_att[64:65, :])
                        psb = ps_bp.tile([64, 512], f32)
                        nc.tensor.matmul(
                            psb[:, :],
                            ones_sb[:, :],
                            rc[:, :],
                            start=True,
                            stop=True,
                        )
                        bc = bcp.tile([64, 512], f32)
                        nc.vector.tensor_copy(bc[:, :], psb[:, :])
                        nc.vector.tensor_mul(
                            attnT[pb : pb + 64, tq, :],
                            ps_att[0:64, :],
                            bc[:, :],
                        )
                    for ss in range(4):  # s sub-tiles of this q chunk
                        for nch in range(2):  # dmodel chunks of 512
                            ps_y = ps_yp.tile([128, 512], f32)
                            for t in range(4):
                                nc.tensor.matmul(
                                    ps_y[:, :],
                                    attnT[:, t, ss * 128 : (ss + 1) * 128],
                                    wo_sb[:, t, nch * 512 : (nch + 1) * 512],
                                    start=(t == 0),
                                    stop=(t == 3),
                                )
                            yt = yp.tile([128, 512], f32)
                            nc.vector.tensor_copy(yt[:, :], ps_y[:, :])
                            nc.gpsimd.dma_start(
                                y_d[
                                    qc * 512 + ss * 128 : qc * 512 + (ss + 1) * 128,
                                    nch * 512 : (nch + 1) * 512,
                                ],
                                yt[:, :],
                            )
    nc.finalize()
    return nc


def _get_nc():
    global _cached
    if _cached is None:
        _cached = _build()
    return _cached


def kernel(x, w_qkv, b_qkv, w_out, b_out, trace=False):
    from concourse.bass_utils import run_bass_kernel_spmd

    x = np.asarray(x, np.float32)
    w_qkv = np.asarray(w_qkv, np.float32)
    b_qkv = np.asarray(b_qkv, np.float32)
    w_out = np.asarray(w_out, np.float32)
    b_out = np.asarray(b_out, np.float32)

    in_maps = []
    for c in range(N_CORES):
        b, g = c // 2, c % 2
        cs = slice(g * DG, (g + 1) * DG)
        # fold the 1/sqrt(HD)=1/8 score scale into the Q projection
        wq = np.ascontiguousarray(w_qkv[:, 0 * D + g * DG : 0 * D + (g + 1) * DG]) / 8.0
        wk = np.ascontiguousarray(w_qkv[:, 1 * D + g * DG : 1 * D + (g + 1) * DG])
        wv = np.ascontiguousarray(w_qkv[:, 2 * D + g * DG : 2 * D + (g + 1) * DG])
        bq = (b_qkv[0 * D + g * DG : 0 * D + (g + 1) * DG] / 8.0).reshape(4, 128).T
        bk = b_qkv[1 * D + g * DG : 1 * D + (g + 1) * DG].reshape(4, 128).T
        bv = b_qkv[2 * D + g * DG : 2 * D + (g + 1) * DG].reshape(1, DG)
        in_maps.append(
            {
                "xT": np.ascontiguousarray(x[b].T),
                "wq": wq,
                "wk": wk,
                "wv": wv,
                "bq": np.ascontiguousarray(bq),
                "bk": np.ascontiguousarray(bk),
                "bv": np.ascontiguousarray(bv),
                "wo": np.ascontiguousarray(w_out[g * DG : (g + 1) * DG, :]),
                "ones": np.ones((1, 64), np.float32),
            }
        )

    nc = _get_nc()
    res = run_bass_kernel_spmd(nc, in_maps, list(range(N_CORES)), trace=trace)
    out = np.empty((B, S, D), np.float32)
    for b in range(B):
        out[b] = res.results[2 * b]["y"] + res.results[2 * b + 1]["y"] + b_out
    if trace:
        kernel.last_exec_time_ns = res.exec_time_ns
        kernel.last_results = res
    return out
